# revision 12
# baseline (speedup 1.0000x reference)
"""Trainium2 Bass kernel for a diagonal-recurrence RNN (BPTT forward).

Computes h = scan(h_t = lamda * h_{t-1} + u_t) with u = x_sequence @ B.T,
for T=8192, H=2048, fp32.

Strategy (8 NeuronCores, SPMD, no collectives):
  - Shard hidden dim H across cores: core c owns units [c*256, (c+1)*256).
  - Host pre-permutes x into per-time-chunk, partition-major blocks
    xC[c, p, kt, t] = x[c*512+t, kt*128+p] so every DMA descriptor is a
    32KB contiguous run per partition (line-rate HBM).
  - GEMM: per 512-wide time chunk, 16 k-tile matmuls accumulate
    u[128h, 512t] in PSUM (float32r: full PE rate, FP22 mantissa).
  - Scan: DVE tensor_tensor_scan reads the PSUM accumulator directly and
    writes h[128h, 512t] to SBUF (fp32 state), chained across chunks via
    the previous chunk's last column.
  - h shards DMA out in [ht, p, t] layout; host reassembles + transposes.
"""

import numpy as np

import concourse.bass as bass
import concourse.mybir as mybir
import concourse.tile as tile
from concourse import bacc
from concourse.bass_utils import run_bass_kernel_spmd

T, H = 8192, 2048
N_CORES = 8
HS = H // N_CORES  # hidden units per core (256)
P = 128  # SBUF partitions
N_HT = HS // P  # hidden partition-tiles per core (2)
N_KT = H // P  # contraction tiles (16)
CHUNK = 512  # time chunk (one PSUM bank of fp32)
N_CHUNKS = T // CHUNK  # 16

# "f32r": fp32 storage, float32r matmul (FP22 mantissa, full PE rate)
# "f32" : true fp32 matmul (1/4 PE rate)
# "f16" : x/B cast to fp16 on host (half DMA, full PE rate)
MM_DTYPE = "f16"

_NC_CACHE = {}


def _dtypes(mm_dtype: str):
    if mm_dtype == "f32r":
        return mybir.dt.float32r, np.float32
    if mm_dtype == "f32":
        return mybir.dt.float32, np.float32
    if mm_dtype == "f16":
        return mybir.dt.float16, np.float16
    if mm_dtype == "bf16":
        import ml_dtypes

        return mybir.dt.bfloat16, np.dtype(ml_dtypes.bfloat16)
    raise ValueError(mm_dtype)


def _build(mm_dtype: str):
    store_dt, np_dt = _dtypes(mm_dtype)
    f32 = mybir.dt.float32

    nc = bacc.Bacc("TRN2", target_bir_lowering=False, debug=False, num_devices=N_CORES)
    xC = nc.dram_tensor("xC", [N_CHUNKS, P, N_KT, CHUNK], store_dt, kind="ExternalInput")
    BT = nc.dram_tensor("BT", [P, N_KT, HS], store_dt, kind="ExternalInput")
    lam = nc.dram_tensor("lam", [N_HT, P], f32, kind="ExternalInput")
    hT = nc.dram_tensor("hT", [N_HT, P, T], f32, kind="ExternalOutput")

    with tile.TileContext(nc) as tc:
        with (
            tc.tile_pool(name="const", bufs=1) as cpool,
            tc.tile_pool(name="xin", bufs=4) as xpool,
            tc.tile_pool(name="hout", bufs=3) as hpool,
            tc.tile_pool(name="ps", bufs=2, space="PSUM") as pspool,
        ):
            # split first loads so the PE can start on k-tiles 0-7 early
            xt0 = xpool.tile([P, N_KT, CHUNK], store_dt, name="xt", tag="xt")
            bt = cpool.tile([P, N_KT, HS], store_dt)
            KH = N_KT // 2
            nc.sync.dma_start(xt0[:, :KH, :], xC.ap()[0, :, :KH, :])
            nc.sync.dma_start(bt[:, :KH, :], BT.ap()[:, :KH, :])
            nc.sync.dma_start(xt0[:, KH:, :], xC.ap()[0, :, KH:, :])
            nc.sync.dma_start(bt[:, KH:, :], BT.ap()[:, KH:, :])
            lam_sb = cpool.tile([P, N_HT], f32)
            nc.sync.dma_start(lam_sb[:], lam.ap().rearrange("h p -> p h"))
            lam_b = cpool.tile([P, N_HT, CHUNK], f32)
            for ht in range(N_HT):
                nc.vector.memset(lam_b[:, ht, :], 1.0)
                nc.vector.tensor_scalar_mul(
                    lam_b[:, ht, :], lam_b[:, ht, :], lam_sb[:, ht : ht + 1]
                )

            # two chunks share one h staging tile -> 4KB store descriptors
            GRP = 2
            prev = [None] * N_HT  # (tile, col) of last written scan column
            for cg in range(N_CHUNKS // GRP):
                hgrp = [None] * N_HT
                for sub in range(GRP):
                    c = cg * GRP + sub
                    if c == 0:
                        xt = xt0
                    else:
                        xt = xpool.tile([P, N_KT, CHUNK], store_dt, tag="xt")
                        nc.sync.dma_start(xt[:], xC.ap()[c])
                    for ht in range(N_HT):
                        ps = pspool.tile([P, CHUNK], f32, tag=f"ps{ht}")
                        for kt in range(N_KT):
                            nc.tensor.matmul(
                                ps[:],
                                bt[:, kt, ht * P : (ht + 1) * P],
                                xt[:, kt, :],
                                start=(kt == 0),
                                stop=(kt == N_KT - 1),
                            )
                        if sub == 0:
                            hgrp[ht] = hpool.tile(
                                [P, GRP * CHUNK], f32, name=f"h{ht}", tag=f"h{ht}"
                            )
                        hseg = hgrp[ht][:, sub * CHUNK : (sub + 1) * CHUNK]
                        initial = (
                            0.0
                            if c == 0
                            else prev[ht][0][:, prev[ht][1] : prev[ht][1] + 1]
                        )
                        nc.vector.tensor_tensor_scan(
                            hseg,
                            lam_b[:, ht, :],
                            ps[:],
                            initial,
                            mybir.AluOpType.mult,
                            mybir.AluOpType.add,
                        )
                        prev[ht] = (hgrp[ht], (sub + 1) * CHUNK - 1)
                        if sub == GRP - 1:
                            # scalar (ACT) HWDGE ring: store issue can't
                            # head-of-line-block the next chunk's load on Sync
                            nc.scalar.dma_start(
                                hT.ap()[ht, :, bass.ts(cg, GRP * CHUNK)], hgrp[ht][:]
                            )
    nc.compile()
    return nc, np_dt


def _get_nc(mm_dtype: str):
    if mm_dtype not in _NC_CACHE:
        _NC_CACHE[mm_dtype] = _build(mm_dtype)
    return _NC_CACHE[mm_dtype]


def kernel(x_sequence, lamda, B, _run_kwargs=None):
    x = np.ascontiguousarray(np.asarray(x_sequence), dtype=np.float32)
    lamda = np.ascontiguousarray(np.asarray(lamda), dtype=np.float32)
    B = np.ascontiguousarray(np.asarray(B), dtype=np.float32)
    assert x.shape == (T, H) and lamda.shape == (H,) and B.shape == (H, H)

    nc, np_dt = _get_nc(MM_DTYPE)

    # xC[c, p, kt, t] = x[c*CHUNK+t, kt*P+p]: per-partition contiguous blocks.
    xC = np.ascontiguousarray(
        x.reshape(N_CHUNKS, CHUNK, N_KT, P).transpose(0, 3, 2, 1).astype(np_dt)
    )
    in_maps = []
    for c in range(N_CORES):
        sl = slice(c * HS, (c + 1) * HS)
        # BT[p, kt, h] = B[core_base + h, kt*P + p]
        BT_c = np.ascontiguousarray(
            B[sl, :].reshape(HS, N_KT, P).transpose(2, 1, 0).astype(np_dt)
        )
        in_maps.append(
            {
                "xC": xC,
                "BT": BT_c,
                "lam": np.ascontiguousarray(lamda[sl].reshape(N_HT, P)),
            }
        )

    res = run_bass_kernel_spmd(
        nc, in_maps, core_ids=list(range(N_CORES)), **(_run_kwargs or {})
    )
    # hT per core: [N_HT, P, T] with h_global[t, c*HS + ht*P + p] = hT[ht, p, t]
    out = np.empty((T, H), dtype=np.float32)
    for c in range(N_CORES):
        hTc = res.results[c]["hT"]  # [N_HT, P, T]
        out[:, c * HS : (c + 1) * HS] = hTc.reshape(HS, T).T
    if _run_kwargs:
        kernel.last_results = res
    return out


# revision 14
# speedup vs baseline: 1.0310x; 1.0310x over previous
"""Trainium2 Bass kernel for a diagonal-recurrence RNN (BPTT forward).

Computes h = scan(h_t = lamda * h_{t-1} + u_t) with u = x_sequence @ B.T,
for T=8192, H=2048, fp32.

Strategy (8 NeuronCores, SPMD, no collectives):
  - Shard hidden dim H across cores: core c owns units [c*256, (c+1)*256).
  - Host pre-permutes x into per-time-chunk, partition-major blocks
    xC[c, p, kt, t] = x[c*512+t, kt*128+p] so every DMA descriptor is a
    32KB contiguous run per partition (line-rate HBM).
  - GEMM: per 512-wide time chunk, 16 k-tile matmuls accumulate
    u[128h, 512t] in PSUM (float32r: full PE rate, FP22 mantissa).
  - Scan: DVE tensor_tensor_scan reads the PSUM accumulator directly and
    writes h[128h, 512t] to SBUF (fp32 state), chained across chunks via
    the previous chunk's last column.
  - h shards DMA out in [ht, p, t] layout; host reassembles + transposes.
"""

import numpy as np

import concourse.bass as bass
import concourse.mybir as mybir
import concourse.tile as tile
from concourse import bacc
from concourse.bass_utils import run_bass_kernel_spmd

T, H = 8192, 2048
N_CORES = 8
HS = H // N_CORES  # hidden units per core (256)
P = 128  # SBUF partitions
N_HT = HS // P  # hidden partition-tiles per core (2)
N_KT = H // P  # contraction tiles (16)
CHUNK = 512  # time chunk (one PSUM bank of fp32)
N_CHUNKS = T // CHUNK  # 16

# "f32r": fp32 storage, float32r matmul (FP22 mantissa, full PE rate)
# "f32" : true fp32 matmul (1/4 PE rate)
# "f16" : x/B cast to fp16 on host (half DMA, full PE rate)
MM_DTYPE = "f16"

_NC_CACHE = {}


def _dtypes(mm_dtype: str):
    if mm_dtype == "f32r":
        return mybir.dt.float32r, np.float32
    if mm_dtype == "f32":
        return mybir.dt.float32, np.float32
    if mm_dtype == "f16":
        return mybir.dt.float16, np.float16
    if mm_dtype == "bf16":
        import ml_dtypes

        return mybir.dt.bfloat16, np.dtype(ml_dtypes.bfloat16)
    raise ValueError(mm_dtype)


def _build(mm_dtype: str):
    store_dt, np_dt = _dtypes(mm_dtype)
    f32 = mybir.dt.float32

    nc = bacc.Bacc("TRN2", target_bir_lowering=False, debug=False, num_devices=N_CORES)
    xC = nc.dram_tensor("xC", [N_CHUNKS, P, N_KT, CHUNK], store_dt, kind="ExternalInput")
    BT = nc.dram_tensor("BT", [P, N_KT, HS], store_dt, kind="ExternalInput")
    lam = nc.dram_tensor("lam", [N_HT, P], f32, kind="ExternalInput")
    hT = nc.dram_tensor("hT", [N_HT, P, T], f32, kind="ExternalOutput")

    with tile.TileContext(nc) as tc:
        with (
            tc.tile_pool(name="const", bufs=1) as cpool,
            tc.tile_pool(name="xin", bufs=4) as xpool,
            tc.tile_pool(name="hout", bufs=3) as hpool,
            tc.tile_pool(name="ps", bufs=2, space="PSUM") as pspool,
        ):
            # split first loads so the PE can start on k-tiles 0-7 early
            xt0 = xpool.tile([P, N_KT, CHUNK], store_dt, name="xt", tag="xt")
            bt = cpool.tile([P, N_KT, HS], store_dt)
            KH = N_KT // 4
            for q in range(4):
                ksl = slice(q * KH, (q + 1) * KH)
                nc.sync.dma_start(xt0[:, ksl, :], xC.ap()[0, :, ksl, :])
                nc.sync.dma_start(bt[:, ksl, :], BT.ap()[:, ksl, :])
            lam_sb = cpool.tile([P, N_HT], f32)
            nc.sync.dma_start(lam_sb[:], lam.ap().rearrange("h p -> p h"))
            lam_b = cpool.tile([P, N_HT, CHUNK], f32)
            for ht in range(N_HT):
                nc.vector.memset(lam_b[:, ht, :], 1.0)
                nc.vector.tensor_scalar_mul(
                    lam_b[:, ht, :], lam_b[:, ht, :], lam_sb[:, ht : ht + 1]
                )

            # two chunks share one h staging tile -> 4KB store descriptors
            GRP = 2
            prev = [None] * N_HT  # (tile, col) of last written scan column
            for cg in range(N_CHUNKS // GRP):
                hgrp = [None] * N_HT
                for sub in range(GRP):
                    c = cg * GRP + sub
                    if c == 0:
                        xt = xt0
                    else:
                        xt = xpool.tile([P, N_KT, CHUNK], store_dt, tag="xt")
                        nc.sync.dma_start(xt[:], xC.ap()[c])
                    for ht in range(N_HT):
                        ps = pspool.tile([P, CHUNK], f32, tag=f"ps{ht}")
                        for kt in range(N_KT):
                            nc.tensor.matmul(
                                ps[:],
                                bt[:, kt, ht * P : (ht + 1) * P],
                                xt[:, kt, :],
                                start=(kt == 0),
                                stop=(kt == N_KT - 1),
                            )
                        if sub == 0:
                            hgrp[ht] = hpool.tile(
                                [P, GRP * CHUNK], f32, name=f"h{ht}", tag=f"h{ht}"
                            )
                        hseg = hgrp[ht][:, sub * CHUNK : (sub + 1) * CHUNK]
                        initial = (
                            0.0
                            if c == 0
                            else prev[ht][0][:, prev[ht][1] : prev[ht][1] + 1]
                        )
                        nc.vector.tensor_tensor_scan(
                            hseg,
                            lam_b[:, ht, :],
                            ps[:],
                            initial,
                            mybir.AluOpType.mult,
                            mybir.AluOpType.add,
                        )
                        prev[ht] = (hgrp[ht], (sub + 1) * CHUNK - 1)
                        # scalar (ACT) HWDGE ring: store issue can't
                        # head-of-line-block the next chunk's load on Sync.
                        # Last group: store each chunk as soon as it's scanned
                        # to shorten the serial tail.
                        if cg == N_CHUNKS // GRP - 1:
                            nc.scalar.dma_start(
                                hT.ap()[ht, :, bass.ts(c, CHUNK)], hseg
                            )
                        elif sub == GRP - 1:
                            nc.scalar.dma_start(
                                hT.ap()[ht, :, bass.ts(cg, GRP * CHUNK)], hgrp[ht][:]
                            )
    nc.compile()
    return nc, np_dt


def _get_nc(mm_dtype: str):
    if mm_dtype not in _NC_CACHE:
        _NC_CACHE[mm_dtype] = _build(mm_dtype)
    return _NC_CACHE[mm_dtype]


def kernel(x_sequence, lamda, B, _run_kwargs=None):
    x = np.ascontiguousarray(np.asarray(x_sequence), dtype=np.float32)
    lamda = np.ascontiguousarray(np.asarray(lamda), dtype=np.float32)
    B = np.ascontiguousarray(np.asarray(B), dtype=np.float32)
    assert x.shape == (T, H) and lamda.shape == (H,) and B.shape == (H, H)

    nc, np_dt = _get_nc(MM_DTYPE)

    # xC[c, p, kt, t] = x[c*CHUNK+t, kt*P+p]: per-partition contiguous blocks.
    xC = np.ascontiguousarray(
        x.reshape(N_CHUNKS, CHUNK, N_KT, P).transpose(0, 3, 2, 1).astype(np_dt)
    )
    in_maps = []
    for c in range(N_CORES):
        sl = slice(c * HS, (c + 1) * HS)
        # BT[p, kt, h] = B[core_base + h, kt*P + p]
        BT_c = np.ascontiguousarray(
            B[sl, :].reshape(HS, N_KT, P).transpose(2, 1, 0).astype(np_dt)
        )
        in_maps.append(
            {
                "xC": xC,
                "BT": BT_c,
                "lam": np.ascontiguousarray(lamda[sl].reshape(N_HT, P)),
            }
        )

    res = run_bass_kernel_spmd(
        nc, in_maps, core_ids=list(range(N_CORES)), **(_run_kwargs or {})
    )
    # hT per core: [N_HT, P, T] with h_global[t, c*HS + ht*P + p] = hT[ht, p, t]
    out = np.empty((T, H), dtype=np.float32)
    for c in range(N_CORES):
        hTc = res.results[c]["hT"]  # [N_HT, P, T]
        out[:, c * HS : (c + 1) * HS] = hTc.reshape(HS, T).T
    if _run_kwargs:
        kernel.last_results = res
    return out


# revision 15
# speedup vs baseline: 1.0876x; 1.0549x over previous
"""Trainium2 Bass kernel for a diagonal-recurrence RNN (BPTT forward).

Computes h = scan(h_t = lamda * h_{t-1} + u_t) with u = x_sequence @ B.T,
for T=8192, H=2048, fp32.

Strategy (8 NeuronCores, SPMD, no collectives):
  - Shard hidden dim H across cores: core c owns units [c*256, (c+1)*256).
  - Host pre-permutes x into per-time-chunk, partition-major blocks
    xC[c, p, kt, t] = x[c*512+t, kt*128+p] so every DMA descriptor is a
    32KB contiguous run per partition (line-rate HBM).
  - GEMM: per 512-wide time chunk, 16 k-tile matmuls accumulate
    u[128h, 512t] in PSUM (float32r: full PE rate, FP22 mantissa).
  - Scan: DVE tensor_tensor_scan reads the PSUM accumulator directly and
    writes h[128h, 512t] to SBUF (fp32 state), chained across chunks via
    the previous chunk's last column.
  - h shards DMA out in [ht, p, t] layout; host reassembles + transposes.
"""

import numpy as np

import concourse.bass as bass
import concourse.mybir as mybir
import concourse.tile as tile
from concourse import bacc
from concourse.bass_utils import run_bass_kernel_spmd

T, H = 8192, 2048
N_CORES = 8
HS = H // N_CORES  # hidden units per core (256)
P = 128  # SBUF partitions
N_HT = HS // P  # hidden partition-tiles per core (2)
N_KT = H // P  # contraction tiles (16)
CHUNK = 512  # time chunk (one PSUM bank of fp32)
N_CHUNKS = T // CHUNK  # 16

# "f32r": fp32 storage, float32r matmul (FP22 mantissa, full PE rate)
# "f32" : true fp32 matmul (1/4 PE rate)
# "f16" : x/B cast to fp16 on host (half DMA, full PE rate)
MM_DTYPE = "f16"

_NC_CACHE = {}


def _dtypes(mm_dtype: str):
    if mm_dtype == "f32r":
        return mybir.dt.float32r, np.float32
    if mm_dtype == "f32":
        return mybir.dt.float32, np.float32
    if mm_dtype == "f16":
        return mybir.dt.float16, np.float16
    if mm_dtype == "bf16":
        import ml_dtypes

        return mybir.dt.bfloat16, np.dtype(ml_dtypes.bfloat16)
    raise ValueError(mm_dtype)


def _build(mm_dtype: str):
    store_dt, np_dt = _dtypes(mm_dtype)
    f32 = mybir.dt.float32

    nc = bacc.Bacc("TRN2", target_bir_lowering=False, debug=False, num_devices=N_CORES)
    xC = nc.dram_tensor("xC", [N_CHUNKS, P, N_KT, CHUNK], store_dt, kind="ExternalInput")
    BT = nc.dram_tensor("BT", [P, N_KT, HS], store_dt, kind="ExternalInput")
    lam = nc.dram_tensor("lam", [N_HT, P], f32, kind="ExternalInput")
    hT = nc.dram_tensor("hT", [N_HT, P, T], f32, kind="ExternalOutput")

    with tile.TileContext(nc) as tc:
        with (
            tc.tile_pool(name="const", bufs=1) as cpool,
            tc.tile_pool(name="xin", bufs=6) as xpool,
            tc.tile_pool(name="hout", bufs=3) as hpool,
            tc.tile_pool(name="ps", bufs=2, space="PSUM") as pspool,
        ):
            # split first loads so the PE can start on k-tiles 0-7 early
            xt0 = xpool.tile([P, N_KT, CHUNK], store_dt, name="xt", tag="xt")
            bt = cpool.tile([P, N_KT, HS], store_dt)
            KH = N_KT // 4
            for q in range(4):
                ksl = slice(q * KH, (q + 1) * KH)
                nc.sync.dma_start(xt0[:, ksl, :], xC.ap()[0, :, ksl, :])
                nc.sync.dma_start(bt[:, ksl, :], BT.ap()[:, ksl, :])
            lam_sb = cpool.tile([P, N_HT], f32)
            nc.sync.dma_start(lam_sb[:], lam.ap().rearrange("h p -> p h"))
            lam_b = cpool.tile([P, N_HT, CHUNK], f32)
            for ht in range(N_HT):
                nc.vector.memset(lam_b[:, ht, :], 1.0)
                nc.vector.tensor_scalar_mul(
                    lam_b[:, ht, :], lam_b[:, ht, :], lam_sb[:, ht : ht + 1]
                )

            # two chunks share one h staging tile -> 4KB store descriptors
            GRP = 2
            prev = [None] * N_HT  # (tile, col) of last written scan column
            for cg in range(N_CHUNKS // GRP):
                hgrp = [None] * N_HT
                for sub in range(GRP):
                    c = cg * GRP + sub
                    if c == 0:
                        xt = xt0
                    else:
                        xt = xpool.tile([P, N_KT, CHUNK], store_dt, tag="xt")
                        nc.sync.dma_start(xt[:], xC.ap()[c])
                    for ht in range(N_HT):
                        ps = pspool.tile([P, CHUNK], f32, tag=f"ps{ht}")
                        for kt in range(N_KT):
                            nc.tensor.matmul(
                                ps[:],
                                bt[:, kt, ht * P : (ht + 1) * P],
                                xt[:, kt, :],
                                start=(kt == 0),
                                stop=(kt == N_KT - 1),
                            )
                        if sub == 0:
                            hgrp[ht] = hpool.tile(
                                [P, GRP * CHUNK], f32, name=f"h{ht}", tag=f"h{ht}"
                            )
                        hseg = hgrp[ht][:, sub * CHUNK : (sub + 1) * CHUNK]
                        initial = (
                            0.0
                            if c == 0
                            else prev[ht][0][:, prev[ht][1] : prev[ht][1] + 1]
                        )
                        nc.vector.tensor_tensor_scan(
                            hseg,
                            lam_b[:, ht, :],
                            ps[:],
                            initial,
                            mybir.AluOpType.mult,
                            mybir.AluOpType.add,
                        )
                        prev[ht] = (hgrp[ht], (sub + 1) * CHUNK - 1)
                        # scalar (ACT) HWDGE ring: store issue can't
                        # head-of-line-block the next chunk's load on Sync.
                        # Last group: store each chunk as soon as it's scanned
                        # to shorten the serial tail.
                        if cg == N_CHUNKS // GRP - 1:
                            nc.scalar.dma_start(
                                hT.ap()[ht, :, bass.ts(c, CHUNK)], hseg
                            )
                        elif sub == GRP - 1:
                            nc.scalar.dma_start(
                                hT.ap()[ht, :, bass.ts(cg, GRP * CHUNK)], hgrp[ht][:]
                            )
    nc.compile()
    return nc, np_dt


def _get_nc(mm_dtype: str):
    if mm_dtype not in _NC_CACHE:
        _NC_CACHE[mm_dtype] = _build(mm_dtype)
    return _NC_CACHE[mm_dtype]


def kernel(x_sequence, lamda, B, _run_kwargs=None):
    x = np.ascontiguousarray(np.asarray(x_sequence), dtype=np.float32)
    lamda = np.ascontiguousarray(np.asarray(lamda), dtype=np.float32)
    B = np.ascontiguousarray(np.asarray(B), dtype=np.float32)
    assert x.shape == (T, H) and lamda.shape == (H,) and B.shape == (H, H)

    nc, np_dt = _get_nc(MM_DTYPE)

    # xC[c, p, kt, t] = x[c*CHUNK+t, kt*P+p]: per-partition contiguous blocks.
    xC = np.ascontiguousarray(
        x.reshape(N_CHUNKS, CHUNK, N_KT, P).transpose(0, 3, 2, 1).astype(np_dt)
    )
    in_maps = []
    for c in range(N_CORES):
        sl = slice(c * HS, (c + 1) * HS)
        # BT[p, kt, h] = B[core_base + h, kt*P + p]
        BT_c = np.ascontiguousarray(
            B[sl, :].reshape(HS, N_KT, P).transpose(2, 1, 0).astype(np_dt)
        )
        in_maps.append(
            {
                "xC": xC,
                "BT": BT_c,
                "lam": np.ascontiguousarray(lamda[sl].reshape(N_HT, P)),
            }
        )

    res = run_bass_kernel_spmd(
        nc, in_maps, core_ids=list(range(N_CORES)), **(_run_kwargs or {})
    )
    # hT per core: [N_HT, P, T] with h_global[t, c*HS + ht*P + p] = hT[ht, p, t]
    out = np.empty((T, H), dtype=np.float32)
    for c in range(N_CORES):
        hTc = res.results[c]["hT"]  # [N_HT, P, T]
        out[:, c * HS : (c + 1) * HS] = hTc.reshape(HS, T).T
    if _run_kwargs:
        kernel.last_results = res
    return out
